# revision 21
# baseline (speedup 1.0000x reference)
"""Trainium2 Bass kernel for a binary (1w1a) depthwise-separable conv block.

Reference computation (NCHW, B=32, C=CO=512, H=W=56):
    xb  = sign(x)
    y1  = depthwise_conv3x3(xb, sign(w_dw), pad=1)          # per-channel
    z   = sign(y1 * s1 + t1)                                # BN1 + binarize
    y2  = pointwise_conv1x1(z, sign(w_pw))                  # dense 512->512
    out = y2 * s2 + t2                                      # BN2

Sharding: data-parallel over batch, 4 images per core on 8 cores.

All intermediate values are {-1, 0, +1}; products and the <=512-term fp32 PSUM
accumulations are exact in fp8/bf16, so the result matches fp32 reference
numerics except for the final BN2 affine (done in fp32) and ~1-ulp BN-constant
rounding.

Device mapping (per core, per image/channel-group):
  - sign(x)          -> ScalarE Sign LUT, fp32 -> fp8 into slot 0 of a
                        zero-bordered [128, 2, 60, 64] buffer (pitch 64).
                        Slot 1 = slot 0 shifted left 2 cols (one DVE bf16
                        copy); all other tap shifts are expressed directly as
                        overlapping access patterns (slot stride 64 = one
                        row), which walrus accepts because 64 % 16 == 0.
  - depthwise conv   -> TensorE fp8 DoubleRow: 5 accumulating passes per
                        8-row output chunk contract the 9 taps: 3 row-pairs
                        (dh=0,dw)+(dh=1,dw) via slot stride 64, the pair
                        (2,0)+(2,2) via the 2-col-shifted slot 1 (stride
                        3840), and (2,1) alone against a zero stationary
                        slot.  Stationary = [128, 2, 128] diagonal pair.
  - BN1 + sign       -> ScalarE: z = Sign(scale*psum + bias) -> fp8, written
                        into [128, 2, 56, 56] z-pair tiles (slot = channel
                        group parity).
  - pointwise conv   -> TensorE fp8 DoubleRow: 2 accumulating passes contract
                        all 512 input channels (2 channel groups per pass).
  - BN2 + evict      -> VectorE tensor_scalar over 2 PSUM banks at a time:
                        psum*s2 + t2 -> fp32 SBUF.
PSUM: one shared pool of 4 x [128, 2, 512] (2-bank) tiles round-robins
between depthwise chunk-pairs and pointwise chunk-pairs (8 banks total).
"""

import sys

sys.path.insert(0, "/opt/trn_rl_repo")

from contextlib import ExitStack

import ml_dtypes
import numpy as np

import concourse.bass as bass
import concourse.tile as tile
from concourse import mybir
from concourse.bass_utils import run_bass_kernel_spmd

N_CORES = 8
B, C, H, W = 32, 512, 56, 56
CO = 512
EPS = 1e-5
BS = B // N_CORES          # images per core
CG = C // 128              # channel groups
ROWS = 8                   # output rows per PSUM bank chunk (8*56=448 fp32)
NCHUNK = H // ROWS         # 7
PH, PW_ = 60, 64           # padded buffer: rows 0/57..59 and cols 0/57..63 zero

F32 = mybir.dt.float32
FP8 = mybir.dt.float8e4
BF16 = mybir.dt.bfloat16
DR = mybir.MatmulPerfMode.DoubleRow
NP_FP8 = ml_dtypes.float8_e4m3

# depthwise passes: (weight pair idx offset, slot stride, row off, col off)
#   p0..p2: taps (0,d)+(1,d)  -- slot stride 64 (next row)
#   p3:     taps (2,0)+(2,2)  -- slot 1 region (2-col-shifted copy), stride 3840
#   p4:     tap  (2,1)+zero   -- zero stationary slot 1, stride 64 reads real
#                                sign data so no NaN can enter the array
DW_PASSES = [(0, 64, 0, 0), (1, 64, 0, 1), (2, 64, 0, 2),
             (3, PH * PW_, 2, 0), (4, 64, 2, 1)]


def _legalize_sem_waits(nc, max_waits=1):
    """walrus (CoreV3 codegen) rejects instructions carrying more than one
    sync-wait command.  Tile's kernel-tail drain waits on every outstanding
    semaphore at once; split excess waits onto preceding no-ops on the same
    engine (engines execute their stream in order, so blocking semantics are
    identical)."""
    n_split = 0
    for f in nc.m.functions:
        for bb in f.blocks:
            insts = bb.instructions
            newlist = []
            for inst in insts:
                si = inst.sync_info
                waits = list(si.on_wait) if si is not None else []
                if len(waits) > max_waits:
                    excess, keep = waits[:-max_waits], waits[-max_waits:]
                    for k, w in enumerate(excess):
                        sp = mybir.InstNoOp(name=f"{inst.name}-lgw{k}")
                        sp.engine = inst.engine
                        sp.sync_info = mybir.SyncInfo(on_wait=[w], on_update=[])
                        newlist.append(sp)
                        n_split += 1
                    inst.sync_info = mybir.SyncInfo(
                        on_wait=keep, on_update=list(si.on_update)
                    )
                newlist.append(inst)
            insts[:] = newlist
    return n_split


def build_bass():
    nc = bass.Bass("TRN2", target_bir_lowering=False, debug=False)

    x_d = nc.dram_tensor("x", [BS, C, H, W], F32, kind="ExternalInput")
    # dw pairs: idx = cg*5 + p; p in 0..2 -> taps (0,p)&(1,p); p=3 -> taps
    # (2,0)&(2,2); p=4 -> tap (2,1) & zero
    wdw_d = nc.dram_tensor("wdw", [128, CG * 5, 2, 128], FP8, kind="ExternalInput")
    # pw pairs: idx = zpair*CG + cob; slot j of zpair holds channels
    # (zpair*2+j)*128 ..
    wpw_d = nc.dram_tensor("wpw", [128, 2 * CG, 2, 128], FP8, kind="ExternalInput")
    bn1_d = nc.dram_tensor("bn1", [128, 2 * CG], F32, kind="ExternalInput")
    bn2_d = nc.dram_tensor("bn2", [128, 2 * CG], F32, kind="ExternalInput")
    y_d = nc.dram_tensor("y", [BS, CO, H, W], F32, kind="ExternalOutput")

    SIGN = mybir.ActivationFunctionType.Sign
    IDENT = mybir.ActivationFunctionType.Identity
    MULT = mybir.AluOpType.mult
    ADD = mybir.AluOpType.add
    IS_GE = mybir.AluOpType.is_ge
    SUB = mybir.AluOpType.subtract

    with tile.TileContext(nc) as tc:
        with ExitStack() as ctx:
            const = ctx.enter_context(tc.tile_pool(name="const", bufs=1))
            xin_pool = ctx.enter_context(tc.tile_pool(name="xin", bufs=7))

            # Image 0 channel-group 0 arrives in three row bands so the first
            # depthwise chunk-group can start as soon as ~18 rows have landed.
            # The cg0 depthwise weights come right after the first band; the
            # bulkier remaining weight/input DMAs follow.
            xin_tiles = {}
            t = xin_pool.tile([128, H, W], F32, tag="xin")
            nc.sync.dma_start(t[:, 0:18, :], x_d.ap()[0, 0:128][:, 0:18, :])
            xin_tiles[(0, 0)] = t

            wdw_t = const.tile([128, CG * 5, 2, 128], FP8, tag="wdw")
            nc.sync.dma_start(wdw_t[:, 0:5], wdw_d.ap()[:, 0:5])

            nc.sync.dma_start(t[:, 18:35, :], x_d.ap()[0, 0:128][:, 18:35, :])
            nc.sync.dma_start(t[:, 35:H, :], x_d.ap()[0, 0:128][:, 35:H, :])

            bn1_t = const.tile([128, 2 * CG], F32, tag="bn1")
            nc.sync.dma_start(bn1_t[:], bn1_d.ap()[:])
            # image 0 cg1 in two bands (its sign stage is banded the same
            # way); remaining tiles whole.  Issue order approximates the
            # order compute consumes the data.
            t = xin_pool.tile([128, H, W], F32, tag="xin")
            nc.sync.dma_start(t[:, 0:35, :], x_d.ap()[0, 128:256][:, 0:35, :])
            nc.sync.dma_start(wdw_t[:, 5:10], wdw_d.ap()[:, 5:10])
            nc.sync.dma_start(t[:, 35:H, :], x_d.ap()[0, 128:256][:, 35:H, :])
            xin_tiles[(0, 1)] = t
            for wcg in range(2, CG):
                nc.sync.dma_start(
                    wdw_t[:, wcg * 5 : (wcg + 1) * 5],
                    wdw_d.ap()[:, wcg * 5 : (wcg + 1) * 5],
                )
            for pcg in range(2, CG):
                t = xin_pool.tile([128, H, W], F32, tag="xin")
                nc.sync.dma_start(t[:], x_d.ap()[0, pcg * 128 : (pcg + 1) * 128])
                xin_tiles[(0, pcg)] = t
            wpw_t = const.tile([128, 2 * CG, 2, 128], FP8, tag="wpw")
            nc.sync.dma_start(wpw_t[:], wpw_d.ap()[:])
            bn2_t = const.tile([128, 2 * CG], F32, tag="bn2")
            nc.sync.dma_start(bn2_t[:], bn2_d.ap()[:])

            # persistent padded sign(x) buffers: [slot, 60, 64].  slot 0 =
            # padded image (borders zero), slot 1 = slot 0 shifted left two
            # columns (rows 2..57 rewritten every use; borders of slot 0 are
            # zeroed once here and never overwritten).
            xpads = []
            for k in range(5):
                xpa = const.tile([128, 2, PH, PW_], FP8, tag=f"xpad{k}")
                xp32 = xpa[:].rearrange("p a b c -> p (a b c)").bitcast(
                    mybir.dt.uint32
                )
                if k == 0:
                    nc.vector.memset(xp32, 0)   # needed immediately
                else:
                    nc.gpsimd.memset(xp32, 0)   # idle engine, needed later
                xpads.append(xpa)

            z_pool = ctx.enter_context(tc.tile_pool(name="z", bufs=4))
            out_pool = ctx.enter_context(tc.tile_pool(name="outb", bufs=4))
            ps_pool = ctx.enter_context(
                tc.tile_pool(name="ps", bufs=4, space="PSUM")
            )

            def mov_ap(xpa, slot_stride, r0, ro, co, nrows=ROWS):
                a = xpa[:]
                v = a.ap
                v[1] = (slot_stride, 2)
                v[2] = (PW_, nrows)
                v[3] = (1, W)
                a.offset = a.offset + (r0 + ro) * PW_ + co
                return a

            prepared = {}

            def prepare(bp, cgp, banded=False):
                """Emit sign(x) + shifted-copy stage for iteration (bp,cgp).
                Called one iteration AHEAD of the matmul consumer; emitted at
                high scheduler priority so the sign runs before same-engine
                z-evictions whenever both are ready (evictions have 3 psum
                groups of slack, the sign gates the next iteration's PE)."""
                itp = bp * CG + cgp
                xin = xin_tiles.pop((bp, cgp))
                xpa = xpads[itp % 5]
                vb = xpa[:].bitcast(BF16)  # [128, 2, 60, 32]
                with tc.high_priority(offset=600):
                    bands = ((0, 35), (35, H)) if banded else ((0, H),)
                    for ra, rb in bands:
                        dst = xpa[:, 0, 1 + ra : 1 + rb, 1 : W + 1]
                        # DVE sign: (x >= 0) - 0.5 = +-0.5; the halved
                        # magnitude is folded into BN1's scale host-side
                        # (still exact fp8 arithmetic).  DVE runs this in
                        # 1.8us vs ScalarE's 2.9us Sign, and it keeps
                        # ScalarE free for the BN1 evictions that gate PSUM
                        # reuse.
                        nc.vector.tensor_scalar(
                            dst, xin[:, ra:rb, :], 0.0, 0.5, IS_GE, SUB
                        )
                    for ra, rb in ((2, 34), (34, 58)) if banded else ((2, 58),):
                        nc.vector.tensor_copy(
                            vb[:, 1, ra:rb, 0:28], vb[:, 0, ra:rb, 1:29]
                        )
                prepared[(bp, cgp)] = xpa

            def dw_psum_group(xpa, cg, pg):
                """Depthwise matmuls + BN1+Sign eviction for one 2-bank PSUM
                chunk-pair.  Pass loop is OUTER so each stationary serves
                both members back-to-back."""
                members = [2 * pg, 2 * pg + 1] if pg < 3 else [6]
                ps2 = ps_pool.tile([128, 2, 512], F32, tag="ps")
                for p, (wi, ss, ro, co) in enumerate(DW_PASSES):
                    for s, n in enumerate(members):
                        nc.tensor.matmul(
                            ps2[:, s, 0 : ROWS * W],
                            wdw_t[:, cg * 5 + wi],
                            mov_ap(xpa, ss, n * ROWS, ro, co),
                            start=(p == 0),
                            stop=(p == 4),
                            perf_mode=DR,
                        )
                return ps2, members

            def dw_evict(ps2, members, zslot, j, cg):
                r0 = members[0] * ROWS
                nrows = ROWS * len(members)
                zout = zslot[:, j, r0 : r0 + nrows, :].rearrange(
                    "p (a r) w -> p a (r w)", a=len(members)
                )
                nc.scalar.activation(
                    zout,
                    ps2[:, 0 : len(members), 0 : ROWS * W],
                    SIGN,
                    bias=bn1_t[:, cg * 2 + 1 : cg * 2 + 2],
                    scale=bn1_t[:, cg * 2 : cg * 2 + 1],
                )

            zp_hist = {}

            def emit_pw_cob(bp, cob, last=False):
                """Pointwise conv + BN2 eviction + output DMA for one block
                of 128 output channels of image bp."""
                zpb = zp_hist[bp]
                outb = out_pool.tile([128, H, W], F32, tag="outb")
                for pg in range(4):
                    members = [2 * pg, 2 * pg + 1] if pg < 3 else [6]
                    pp = ps_pool.tile([128, 2, 512], F32, tag="ps")
                    for zpair in range(2):
                        for s, n in enumerate(members):
                            r0 = n * ROWS
                            nc.tensor.matmul(
                                pp[:, s, 0 : ROWS * W],
                                wpw_t[:, zpair * CG + cob],
                                zpb[zpair][:, :, r0 : r0 + ROWS, :],
                                start=(zpair == 0),
                                stop=(zpair == 1),
                                perf_mode=DR,
                            )
                    r0 = members[0] * ROWS
                    nrows = ROWS * len(members)
                    evicts = [(0, len(members))]
                    if last and pg == 3:
                        # final chunk: single-bank eviction, shortest tail
                        evicts = [(k, k + 1) for k in range(len(members))]
                    for ka, kb in evicts:
                        oout = outb[
                            :, r0 + ka * ROWS : r0 + kb * ROWS, :
                        ].rearrange("p (a r) w -> p a (r w)", a=kb - ka)
                        if last and (pg + ka) % 2 == 1:
                            # ScalarE is idle during the final image's
                            # pointwise (no more BN1 evicts); Identity LUT
                            # computes scale*psum + bias exactly and shaves
                            # the end-of-kernel eviction lag off the tail
                            nc.scalar.activation(
                                oout,
                                pp[:, ka:kb, 0 : ROWS * W],
                                IDENT,
                                bias=bn2_t[:, cob * 2 + 1 : cob * 2 + 2],
                                scale=bn2_t[:, cob * 2 : cob * 2 + 1],
                            )
                        else:
                            nc.vector.tensor_scalar(
                                oout,
                                pp[:, ka:kb, 0 : ROWS * W],
                                bn2_t[:, cob * 2 : cob * 2 + 1],
                                bn2_t[:, cob * 2 + 1 : cob * 2 + 2],
                                MULT,
                                ADD,
                            )
                    # stream rows 0:32 out once chunks 0..3 are evicted
                    if pg == 1:
                        nc_half = y_d.ap()[bp, cob * 128 : (cob + 1) * 128]
                        nc.sync.dma_start(nc_half[:, 0:32, :], outb[:, 0:32, :])
                    if last and pg == 2:
                        nc_half = y_d.ap()[bp, cob * 128 : (cob + 1) * 128]
                        nc.sync.dma_start(nc_half[:, 32:48, :], outb[:, 32:48, :])
                tail = y_d.ap()[bp, cob * 128 : (cob + 1) * 128]
                if last:
                    nc.sync.dma_start(tail[:, 48:H, :], outb[:, 48:H, :])
                else:
                    nc.sync.dma_start(tail[:, 32:H, :], outb[:, 32:H, :])

            # ---- iteration 0: staged sign/copy so the PE starts early ----
            xin0 = xin_tiles.pop((0, 0))
            xpa0 = xpads[0]
            vb0 = xpa0[:].bitcast(BF16)

            def sign_band(ra, rb):
                nc.vector.tensor_scalar(
                    xpa0[:, 0, 1 + ra : 1 + rb, 1 : W + 1],
                    xin0[:, ra:rb, :], 0.0, 0.5, IS_GE, SUB,
                )

            def copy_band(ra, rb):
                nc.vector.tensor_copy(
                    vb0[:, 1, ra:rb, 0:28], vb0[:, 0, ra:rb, 1:29]
                )

            zp0 = []
            for _zi in range(2):
                ztile = z_pool.tile([128, 2, H, W], FP8, tag="z")
                zp0.append(ztile)
            zp_hist[0] = zp0

            sign_band(0, 18)      # Ppad rows 1..18
            copy_band(2, 18)      # slot1 rows 2..17
            g0 = dw_psum_group(xpa0, 0, 0)
            sign_band(18, 35)     # Ppad rows 19..35
            copy_band(18, 34)
            g1 = dw_psum_group(xpa0, 0, 1)
            dw_evict(*g0, zp0[0], 0, 0)
            sign_band(35, H)      # Ppad rows 36..56
            copy_band(34, 58)
            prepare(0, 1, banded=True)  # next iteration, 2-band DMA
            g2 = dw_psum_group(xpa0, 0, 2)
            dw_evict(*g1, zp0[0], 0, 0)
            g3 = dw_psum_group(xpa0, 0, 3)
            dw_evict(*g2, zp0[0], 0, 0)
            dw_evict(*g3, zp0[0], 0, 0)

            next_prep = 2   # (0,0) and (0,1) already staged
            it = 1
            for b in range(BS):
                if b == 0:
                    cg_list = range(1, CG)
                else:
                    cg_list = range(CG)
                    # prefetch next image's inputs ahead of this image's
                    # outputs (output DMA issue blocks on BN2 evictions)
                    zp = []
                    for _zi in range(2):
                        ztile = z_pool.tile([128, 2, H, W], FP8, tag="z")
                        zp.append(ztile)
                    zp_hist[b] = zp
                if b + 1 < BS:
                    for pcg in range(CG):
                        t = xin_pool.tile([128, H, W], F32, tag="xin")
                        nc.sync.dma_start(
                            t[:], x_d.ap()[b + 1, pcg * 128 : (pcg + 1) * 128]
                        )
                        xin_tiles[(b + 1, pcg)] = t
                zp = zp_hist[b]
                for cg in cg_list:
                    it += 1
                    # keep TWO iterations of sign/copy staged ahead of the
                    # matmuls: absorbs input-DMA and ScalarE hiccups without
                    # draining the PE
                    while next_prep <= min(it + 2, BS * CG - 1):
                        prepare(next_prep // CG, next_prep % CG)
                        next_prep += 1
                    xpa = prepared.pop((b, cg))
                    zslot, j = zp[cg // 2], cg % 2
                    for pg in range(4):
                        g = dw_psum_group(xpa, cg, pg)
                        dw_evict(*g, zslot, j, cg)

                    if b > 0:
                        # one block of the previous image's pointwise conv
                        # after each depthwise channel group: spreads the
                        # BN2 evictions so neither DVE nor PSUM reuse ever
                        # bursts, and PE alternates dense work
                        emit_pw_cob(b - 1, cg)
            zp_hist.pop(BS - 2, None)
            for cob in range(CG):
                emit_pw_cob(BS - 1, cob, last=True)
            zp_hist.pop(BS - 1, None)

    _legalize_sem_waits(nc)
    return nc


_NC_CACHE = None


def _get_nc():
    global _NC_CACHE
    if _NC_CACHE is None:
        _NC_CACHE = build_bass()
    return _NC_CACHE


def make_host_inputs(w_dw, w_pw, g1, b1, m1, v1, g2, b2, m2, v2):
    """Host-side preprocessing shared by all cores (weights/BN constants)."""
    wsign = np.sign(w_dw[:, 0, :, :]).reshape(C, 3, 3).astype(np.float32)

    wdw = np.zeros((128, CG * 5, 2, 128), dtype=NP_FP8)
    idx = np.arange(128)
    for cg in range(CG):
        cs = slice(cg * 128, (cg + 1) * 128)
        for dw in range(3):
            wdw[idx, cg * 5 + dw, 0, idx] = wsign[cs, 0, dw].astype(NP_FP8)
            wdw[idx, cg * 5 + dw, 1, idx] = wsign[cs, 1, dw].astype(NP_FP8)
        # pair 3: slot0 = tap (2,0), slot1 = tap (2,2)
        wdw[idx, cg * 5 + 3, 0, idx] = wsign[cs, 2, 0].astype(NP_FP8)
        wdw[idx, cg * 5 + 3, 1, idx] = wsign[cs, 2, 2].astype(NP_FP8)
        # pair 4: slot0 = tap (2,1), slot1 stays zero
        wdw[idx, cg * 5 + 4, 0, idx] = wsign[cs, 2, 1].astype(NP_FP8)

    wptT = np.sign(w_pw[:, :, 0, 0]).T.astype(np.float32)  # [c, co]
    wpw = np.zeros((128, 2 * CG, 2, 128), dtype=NP_FP8)
    for zpair in range(2):
        for cob in range(CG):
            for j in range(2):
                c0 = (zpair * 2 + j) * 128
                wpw[:, zpair * CG + cob, j, :] = wptT[
                    c0 : c0 + 128, cob * 128 : (cob + 1) * 128
                ].astype(NP_FP8)

    def bn_consts(g, bta, m, v):
        s = (g.astype(np.float64) / np.sqrt(v.astype(np.float64) + EPS)).astype(
            np.float32
        )
        t = bta.astype(np.float32) - m.astype(np.float32) * s
        out = np.zeros((128, 2 * CG), dtype=np.float32)
        for cg in range(CG):
            out[:, cg * 2] = s[cg * 128 : (cg + 1) * 128]
            out[:, cg * 2 + 1] = t[cg * 128 : (cg + 1) * 128]
        return out

    bn1 = bn_consts(g1, b1, m1, v1)
    # inputs are sign-binarized on VectorE to +-0.5 instead of +-1; the
    # depthwise PSUM comes out halved, compensated here
    bn1[:, 0::2] *= 2.0
    return {
        "wdw": wdw,
        "wpw": wpw,
        "bn1": bn1,
        "bn2": bn_consts(g2, b2, m2, v2),
    }


def kernel(x, w_dw, w_pw, g1, b1, m1, v1, g2, b2, m2, v2, _trace=False, _tmpdir=None):
    x = np.asarray(x, dtype=np.float32)
    shared = make_host_inputs(
        np.asarray(w_dw), np.asarray(w_pw),
        np.asarray(g1), np.asarray(b1), np.asarray(m1), np.asarray(v1),
        np.asarray(g2), np.asarray(b2), np.asarray(m2), np.asarray(v2),
    )
    in_maps = []
    for i in range(N_CORES):
        m = {"x": np.ascontiguousarray(x[i * BS : (i + 1) * BS])}
        m.update(shared)
        in_maps.append(m)

    nc = _get_nc()
    res = run_bass_kernel_spmd(
        nc, in_maps, core_ids=list(range(N_CORES)), trace=_trace, tmpdir=_tmpdir
    )
    y = np.concatenate([res.results[i]["y"] for i in range(N_CORES)], axis=0)
    if _trace:
        return y, res
    return y


# revision 22
# speedup vs baseline: 1.0158x; 1.0158x over previous
"""Trainium2 Bass kernel for a binary (1w1a) depthwise-separable conv block.

Reference computation (NCHW, B=32, C=CO=512, H=W=56):
    xb  = sign(x)
    y1  = depthwise_conv3x3(xb, sign(w_dw), pad=1)          # per-channel
    z   = sign(y1 * s1 + t1)                                # BN1 + binarize
    y2  = pointwise_conv1x1(z, sign(w_pw))                  # dense 512->512
    out = y2 * s2 + t2                                      # BN2

Sharding: data-parallel over batch, 4 images per core on 8 cores.

All intermediate values are {-1, 0, +1}; products and the <=512-term fp32 PSUM
accumulations are exact in fp8/bf16, so the result matches fp32 reference
numerics except for the final BN2 affine (done in fp32) and ~1-ulp BN-constant
rounding.

Device mapping (per core, per image/channel-group):
  - sign(x)          -> ScalarE Sign LUT, fp32 -> fp8 into slot 0 of a
                        zero-bordered [128, 2, 60, 64] buffer (pitch 64).
                        Slot 1 = slot 0 shifted left 2 cols (one DVE bf16
                        copy); all other tap shifts are expressed directly as
                        overlapping access patterns (slot stride 64 = one
                        row), which walrus accepts because 64 % 16 == 0.
  - depthwise conv   -> TensorE fp8 DoubleRow: 5 accumulating passes per
                        8-row output chunk contract the 9 taps: 3 row-pairs
                        (dh=0,dw)+(dh=1,dw) via slot stride 64, the pair
                        (2,0)+(2,2) via the 2-col-shifted slot 1 (stride
                        3840), and (2,1) alone against a zero stationary
                        slot.  Stationary = [128, 2, 128] diagonal pair.
  - BN1 + sign       -> ScalarE: z = Sign(scale*psum + bias) -> fp8, written
                        into [128, 2, 56, 56] z-pair tiles (slot = channel
                        group parity).
  - pointwise conv   -> TensorE fp8 DoubleRow: 2 accumulating passes contract
                        all 512 input channels (2 channel groups per pass).
  - BN2 + evict      -> VectorE tensor_scalar over 2 PSUM banks at a time:
                        psum*s2 + t2 -> fp32 SBUF.
PSUM: one shared pool of 4 x [128, 2, 512] (2-bank) tiles round-robins
between depthwise chunk-pairs and pointwise chunk-pairs (8 banks total).
"""

import sys

sys.path.insert(0, "/opt/trn_rl_repo")

from contextlib import ExitStack

import ml_dtypes
import numpy as np

import concourse.bass as bass
import concourse.tile as tile
from concourse import mybir
from concourse.bass_utils import run_bass_kernel_spmd

N_CORES = 8
B, C, H, W = 32, 512, 56, 56
CO = 512
EPS = 1e-5
BS = B // N_CORES          # images per core
CG = C // 128              # channel groups
ROWS = 8                   # output rows per PSUM bank chunk (8*56=448 fp32)
NCHUNK = H // ROWS         # 7
PH, PW_ = 60, 64           # padded buffer: rows 0/57..59 and cols 0/57..63 zero

F32 = mybir.dt.float32
FP8 = mybir.dt.float8e4
BF16 = mybir.dt.bfloat16
DR = mybir.MatmulPerfMode.DoubleRow
NP_FP8 = ml_dtypes.float8_e4m3

# depthwise passes: (weight pair idx offset, slot stride, row off, col off)
#   p0..p2: taps (0,d)+(1,d)  -- slot stride 64 (next row)
#   p3:     taps (2,0)+(2,2)  -- slot 1 region (2-col-shifted copy), stride 3840
#   p4:     tap  (2,1)+zero   -- zero stationary slot 1, stride 64 reads real
#                                sign data so no NaN can enter the array
DW_PASSES = [(0, 64, 0, 0), (1, 64, 0, 1), (2, 64, 0, 2),
             (3, PH * PW_, 2, 0), (4, 64, 2, 1)]


def _legalize_sem_waits(nc, max_waits=1):
    """walrus (CoreV3 codegen) rejects instructions carrying more than one
    sync-wait command.  Tile's kernel-tail drain waits on every outstanding
    semaphore at once; split excess waits onto preceding no-ops on the same
    engine (engines execute their stream in order, so blocking semantics are
    identical)."""
    n_split = 0
    for f in nc.m.functions:
        for bb in f.blocks:
            insts = bb.instructions
            newlist = []
            for inst in insts:
                si = inst.sync_info
                waits = list(si.on_wait) if si is not None else []
                if len(waits) > max_waits:
                    excess, keep = waits[:-max_waits], waits[-max_waits:]
                    for k, w in enumerate(excess):
                        sp = mybir.InstNoOp(name=f"{inst.name}-lgw{k}")
                        sp.engine = inst.engine
                        sp.sync_info = mybir.SyncInfo(on_wait=[w], on_update=[])
                        newlist.append(sp)
                        n_split += 1
                    inst.sync_info = mybir.SyncInfo(
                        on_wait=keep, on_update=list(si.on_update)
                    )
                newlist.append(inst)
            insts[:] = newlist
    return n_split


def build_bass():
    nc = bass.Bass("TRN2", target_bir_lowering=False, debug=False)

    x_d = nc.dram_tensor("x", [BS, C, H, W], F32, kind="ExternalInput")
    # dw pairs: idx = cg*5 + p; p in 0..2 -> taps (0,p)&(1,p); p=3 -> taps
    # (2,0)&(2,2); p=4 -> tap (2,1) & zero
    wdw_d = nc.dram_tensor("wdw", [128, CG * 5, 2, 128], FP8, kind="ExternalInput")
    # pw pairs: idx = zpair*CG + cob; slot j of zpair holds channels
    # (zpair*2+j)*128 ..
    wpw_d = nc.dram_tensor("wpw", [128, 2 * CG, 2, 128], FP8, kind="ExternalInput")
    bn1_d = nc.dram_tensor("bn1", [128, 2 * CG], F32, kind="ExternalInput")
    bn2_d = nc.dram_tensor("bn2", [128, 2 * CG], F32, kind="ExternalInput")
    y_d = nc.dram_tensor("y", [BS, CO, H, W], F32, kind="ExternalOutput")

    SIGN = mybir.ActivationFunctionType.Sign
    MULT = mybir.AluOpType.mult
    ADD = mybir.AluOpType.add
    IS_GE = mybir.AluOpType.is_ge
    SUB = mybir.AluOpType.subtract

    with tile.TileContext(nc) as tc:
        with ExitStack() as ctx:
            const = ctx.enter_context(tc.tile_pool(name="const", bufs=1))
            xin_pool = ctx.enter_context(tc.tile_pool(name="xin", bufs=7))

            # Image 0 channel-group 0 arrives in three row bands so the first
            # depthwise chunk-group can start as soon as ~18 rows have landed.
            # The cg0 depthwise weights come right after the first band; the
            # bulkier remaining weight/input DMAs follow.
            xin_tiles = {}
            wdw_t = const.tile([128, CG * 5, 2, 128], FP8, tag="wdw")
            nc.sync.dma_start(wdw_t[:, 0:5], wdw_d.ap()[:, 0:5])

            t = xin_pool.tile([128, H, W], F32, tag="xin")
            nc.sync.dma_start(t[:, 0:18, :], x_d.ap()[0, 0:128][:, 0:18, :])
            xin_tiles[(0, 0)] = t

            nc.sync.dma_start(t[:, 18:35, :], x_d.ap()[0, 0:128][:, 18:35, :])
            nc.sync.dma_start(t[:, 35:H, :], x_d.ap()[0, 0:128][:, 35:H, :])

            bn1_t = const.tile([128, 2 * CG], F32, tag="bn1")
            nc.sync.dma_start(bn1_t[:], bn1_d.ap()[:])
            # image 0 cg1 in two bands (its sign stage is banded the same
            # way); remaining tiles whole.  Issue order approximates the
            # order compute consumes the data.
            t = xin_pool.tile([128, H, W], F32, tag="xin")
            nc.sync.dma_start(t[:, 0:35, :], x_d.ap()[0, 128:256][:, 0:35, :])
            nc.sync.dma_start(wdw_t[:, 5:10], wdw_d.ap()[:, 5:10])
            nc.sync.dma_start(t[:, 35:H, :], x_d.ap()[0, 128:256][:, 35:H, :])
            xin_tiles[(0, 1)] = t
            for wcg in range(2, CG):
                nc.sync.dma_start(
                    wdw_t[:, wcg * 5 : (wcg + 1) * 5],
                    wdw_d.ap()[:, wcg * 5 : (wcg + 1) * 5],
                )
            for pcg in range(2, CG):
                t = xin_pool.tile([128, H, W], F32, tag="xin")
                nc.sync.dma_start(t[:], x_d.ap()[0, pcg * 128 : (pcg + 1) * 128])
                xin_tiles[(0, pcg)] = t
            wpw_t = const.tile([128, 2 * CG, 2, 128], FP8, tag="wpw")
            nc.sync.dma_start(wpw_t[:], wpw_d.ap()[:])
            bn2_t = const.tile([128, 2 * CG], F32, tag="bn2")
            nc.sync.dma_start(bn2_t[:], bn2_d.ap()[:])

            # persistent padded sign(x) buffers: [slot, 60, 64].  slot 0 =
            # padded image (borders zero), slot 1 = slot 0 shifted left two
            # columns (rows 2..57 rewritten every use; borders of slot 0 are
            # zeroed once here and never overwritten).
            xpads = []
            for k in range(5):
                xpa = const.tile([128, 2, PH, PW_], FP8, tag=f"xpad{k}")
                xp32 = xpa[:].rearrange("p a b c -> p (a b c)").bitcast(
                    mybir.dt.uint32
                )
                if k == 0:
                    nc.vector.memset(xp32, 0)   # needed immediately
                else:
                    nc.gpsimd.memset(xp32, 0)   # idle engine, needed later
                xpads.append(xpa)

            z_pool = ctx.enter_context(tc.tile_pool(name="z", bufs=4))
            out_pool = ctx.enter_context(tc.tile_pool(name="outb", bufs=4))
            ps_pool = ctx.enter_context(
                tc.tile_pool(name="ps", bufs=4, space="PSUM")
            )

            def mov_ap(xpa, slot_stride, r0, ro, co, nrows=ROWS):
                a = xpa[:]
                v = a.ap
                v[1] = (slot_stride, 2)
                v[2] = (PW_, nrows)
                v[3] = (1, W)
                a.offset = a.offset + (r0 + ro) * PW_ + co
                return a

            prepared = {}

            def prepare(bp, cgp, banded=False):
                """Emit sign(x) + shifted-copy stage for iteration (bp,cgp).
                Called one iteration AHEAD of the matmul consumer; emitted at
                high scheduler priority so the sign runs before same-engine
                z-evictions whenever both are ready (evictions have 3 psum
                groups of slack, the sign gates the next iteration's PE)."""
                itp = bp * CG + cgp
                xin = xin_tiles.pop((bp, cgp))
                xpa = xpads[itp % 5]
                vb = xpa[:].bitcast(BF16)  # [128, 2, 60, 32]
                with tc.high_priority(offset=200):
                    bands = ((0, 35), (35, H)) if banded else ((0, H),)
                    for ra, rb in bands:
                        dst = xpa[:, 0, 1 + ra : 1 + rb, 1 : W + 1]
                        # DVE sign: (x >= 0) - 0.5 = +-0.5; the halved
                        # magnitude is folded into BN1's scale host-side
                        # (still exact fp8 arithmetic).  DVE runs this in
                        # 1.8us vs ScalarE's 2.9us Sign, and it keeps
                        # ScalarE free for the BN1 evictions that gate PSUM
                        # reuse.
                        nc.vector.tensor_scalar(
                            dst, xin[:, ra:rb, :], 0.0, 0.5, IS_GE, SUB
                        )
                    for ra, rb in ((2, 34), (34, 58)) if banded else ((2, 58),):
                        nc.vector.tensor_copy(
                            vb[:, 1, ra:rb, 0:28], vb[:, 0, ra:rb, 1:29]
                        )
                prepared[(bp, cgp)] = xpa

            def dw_psum_group(xpa, cg, pg):
                """Depthwise matmuls + BN1+Sign eviction for one 2-bank PSUM
                chunk-pair.  Pass loop is OUTER so each stationary serves
                both members back-to-back."""
                members = [2 * pg, 2 * pg + 1] if pg < 3 else [6]
                ps2 = ps_pool.tile([128, 2, 512], F32, tag="ps")
                for p, (wi, ss, ro, co) in enumerate(DW_PASSES):
                    for s, n in enumerate(members):
                        nc.tensor.matmul(
                            ps2[:, s, 0 : ROWS * W],
                            wdw_t[:, cg * 5 + wi],
                            mov_ap(xpa, ss, n * ROWS, ro, co),
                            start=(p == 0),
                            stop=(p == 4),
                            perf_mode=DR,
                        )
                return ps2, members

            def dw_evict(ps2, members, zslot, j, cg):
                r0 = members[0] * ROWS
                nrows = ROWS * len(members)
                zout = zslot[:, j, r0 : r0 + nrows, :].rearrange(
                    "p (a r) w -> p a (r w)", a=len(members)
                )
                nc.scalar.activation(
                    zout,
                    ps2[:, 0 : len(members), 0 : ROWS * W],
                    SIGN,
                    bias=bn1_t[:, cg * 2 + 1 : cg * 2 + 2],
                    scale=bn1_t[:, cg * 2 : cg * 2 + 1],
                )

            zp_hist = {}

            def emit_pw_cob(bp, cob, last=False):
                """Pointwise conv + BN2 eviction + output DMA for one block
                of 128 output channels of image bp."""
                zpb = zp_hist[bp]
                outb = out_pool.tile([128, H, W], F32, tag="outb")
                for pg in range(4):
                    members = [2 * pg, 2 * pg + 1] if pg < 3 else [6]
                    pp = ps_pool.tile([128, 2, 512], F32, tag="ps")
                    for zpair in range(2):
                        for s, n in enumerate(members):
                            r0 = n * ROWS
                            nc.tensor.matmul(
                                pp[:, s, 0 : ROWS * W],
                                wpw_t[:, zpair * CG + cob],
                                zpb[zpair][:, :, r0 : r0 + ROWS, :],
                                start=(zpair == 0),
                                stop=(zpair == 1),
                                perf_mode=DR,
                            )
                    r0 = members[0] * ROWS
                    nrows = ROWS * len(members)
                    evicts = [(0, len(members))]
                    if last and pg == 3:
                        # final chunk: single-bank eviction, shortest tail
                        evicts = [(k, k + 1) for k in range(len(members))]
                    for ka, kb in evicts:
                        oout = outb[
                            :, r0 + ka * ROWS : r0 + kb * ROWS, :
                        ].rearrange("p (a r) w -> p a (r w)", a=kb - ka)
                        nc.vector.tensor_scalar(
                            oout,
                            pp[:, ka:kb, 0 : ROWS * W],
                            bn2_t[:, cob * 2 : cob * 2 + 1],
                            bn2_t[:, cob * 2 + 1 : cob * 2 + 2],
                            MULT,
                            ADD,
                        )
                    # stream rows 0:32 out once chunks 0..3 are evicted
                    if pg == 1:
                        nc_half = y_d.ap()[bp, cob * 128 : (cob + 1) * 128]
                        nc.sync.dma_start(nc_half[:, 0:32, :], outb[:, 0:32, :])
                    if last and pg == 2:
                        nc_half = y_d.ap()[bp, cob * 128 : (cob + 1) * 128]
                        nc.sync.dma_start(nc_half[:, 32:48, :], outb[:, 32:48, :])
                tail = y_d.ap()[bp, cob * 128 : (cob + 1) * 128]
                if last:
                    nc.sync.dma_start(tail[:, 48:H, :], outb[:, 48:H, :])
                else:
                    nc.sync.dma_start(tail[:, 32:H, :], outb[:, 32:H, :])

            # ---- iteration 0: staged sign/copy so the PE starts early ----
            xin0 = xin_tiles.pop((0, 0))
            xpa0 = xpads[0]
            vb0 = xpa0[:].bitcast(BF16)

            def sign_band(ra, rb):
                nc.vector.tensor_scalar(
                    xpa0[:, 0, 1 + ra : 1 + rb, 1 : W + 1],
                    xin0[:, ra:rb, :], 0.0, 0.5, IS_GE, SUB,
                )

            def copy_band(ra, rb):
                nc.vector.tensor_copy(
                    vb0[:, 1, ra:rb, 0:28], vb0[:, 0, ra:rb, 1:29]
                )

            zp0 = []
            for _zi in range(2):
                ztile = z_pool.tile([128, 2, H, W], FP8, tag="z")
                zp0.append(ztile)
            zp_hist[0] = zp0

            sign_band(0, 18)      # Ppad rows 1..18
            copy_band(2, 18)      # slot1 rows 2..17
            g0 = dw_psum_group(xpa0, 0, 0)
            sign_band(18, 35)     # Ppad rows 19..35
            copy_band(18, 34)
            g1 = dw_psum_group(xpa0, 0, 1)
            dw_evict(*g0, zp0[0], 0, 0)
            sign_band(35, H)      # Ppad rows 36..56
            copy_band(34, 58)
            prepare(0, 1, banded=True)  # next iteration, 2-band DMA
            g2 = dw_psum_group(xpa0, 0, 2)
            dw_evict(*g1, zp0[0], 0, 0)
            g3 = dw_psum_group(xpa0, 0, 3)
            dw_evict(*g2, zp0[0], 0, 0)
            dw_evict(*g3, zp0[0], 0, 0)

            next_prep = 2   # (0,0) and (0,1) already staged
            it = 1
            for b in range(BS):
                if b == 0:
                    cg_list = range(1, CG)
                else:
                    cg_list = range(CG)
                    # prefetch next image's inputs ahead of this image's
                    # outputs (output DMA issue blocks on BN2 evictions)
                    zp = []
                    for _zi in range(2):
                        ztile = z_pool.tile([128, 2, H, W], FP8, tag="z")
                        zp.append(ztile)
                    zp_hist[b] = zp
                if b + 1 < BS:
                    for pcg in range(CG):
                        t = xin_pool.tile([128, H, W], F32, tag="xin")
                        nc.sync.dma_start(
                            t[:], x_d.ap()[b + 1, pcg * 128 : (pcg + 1) * 128]
                        )
                        xin_tiles[(b + 1, pcg)] = t
                zp = zp_hist[b]
                for cg in cg_list:
                    it += 1
                    # keep TWO iterations of sign/copy staged ahead of the
                    # matmuls: absorbs input-DMA and ScalarE hiccups without
                    # draining the PE
                    while next_prep <= min(it + 2, BS * CG - 1):
                        prepare(next_prep // CG, next_prep % CG)
                        next_prep += 1
                    xpa = prepared.pop((b, cg))
                    zslot, j = zp[cg // 2], cg % 2
                    for pg in range(4):
                        g = dw_psum_group(xpa, cg, pg)
                        dw_evict(*g, zslot, j, cg)

                    if b > 0:
                        # one block of the previous image's pointwise conv
                        # after each depthwise channel group: spreads the
                        # BN2 evictions so neither DVE nor PSUM reuse ever
                        # bursts, and PE alternates dense work
                        emit_pw_cob(b - 1, cg)
            zp_hist.pop(BS - 2, None)
            for cob in range(CG):
                emit_pw_cob(BS - 1, cob, last=True)
            zp_hist.pop(BS - 1, None)

    _legalize_sem_waits(nc)
    return nc


_NC_CACHE = None


def _get_nc():
    global _NC_CACHE
    if _NC_CACHE is None:
        _NC_CACHE = build_bass()
    return _NC_CACHE


def make_host_inputs(w_dw, w_pw, g1, b1, m1, v1, g2, b2, m2, v2):
    """Host-side preprocessing shared by all cores (weights/BN constants)."""
    wsign = np.sign(w_dw[:, 0, :, :]).reshape(C, 3, 3).astype(np.float32)

    wdw = np.zeros((128, CG * 5, 2, 128), dtype=NP_FP8)
    idx = np.arange(128)
    for cg in range(CG):
        cs = slice(cg * 128, (cg + 1) * 128)
        for dw in range(3):
            wdw[idx, cg * 5 + dw, 0, idx] = wsign[cs, 0, dw].astype(NP_FP8)
            wdw[idx, cg * 5 + dw, 1, idx] = wsign[cs, 1, dw].astype(NP_FP8)
        # pair 3: slot0 = tap (2,0), slot1 = tap (2,2)
        wdw[idx, cg * 5 + 3, 0, idx] = wsign[cs, 2, 0].astype(NP_FP8)
        wdw[idx, cg * 5 + 3, 1, idx] = wsign[cs, 2, 2].astype(NP_FP8)
        # pair 4: slot0 = tap (2,1), slot1 stays zero
        wdw[idx, cg * 5 + 4, 0, idx] = wsign[cs, 2, 1].astype(NP_FP8)

    wptT = np.sign(w_pw[:, :, 0, 0]).T.astype(np.float32)  # [c, co]
    wpw = np.zeros((128, 2 * CG, 2, 128), dtype=NP_FP8)
    for zpair in range(2):
        for cob in range(CG):
            for j in range(2):
                c0 = (zpair * 2 + j) * 128
                wpw[:, zpair * CG + cob, j, :] = wptT[
                    c0 : c0 + 128, cob * 128 : (cob + 1) * 128
                ].astype(NP_FP8)

    def bn_consts(g, bta, m, v):
        s = (g.astype(np.float64) / np.sqrt(v.astype(np.float64) + EPS)).astype(
            np.float32
        )
        t = bta.astype(np.float32) - m.astype(np.float32) * s
        out = np.zeros((128, 2 * CG), dtype=np.float32)
        for cg in range(CG):
            out[:, cg * 2] = s[cg * 128 : (cg + 1) * 128]
            out[:, cg * 2 + 1] = t[cg * 128 : (cg + 1) * 128]
        return out

    bn1 = bn_consts(g1, b1, m1, v1)
    # inputs are sign-binarized on VectorE to +-0.5 instead of +-1; the
    # depthwise PSUM comes out halved, compensated here
    bn1[:, 0::2] *= 2.0
    return {
        "wdw": wdw,
        "wpw": wpw,
        "bn1": bn1,
        "bn2": bn_consts(g2, b2, m2, v2),
    }


def kernel(x, w_dw, w_pw, g1, b1, m1, v1, g2, b2, m2, v2, _trace=False, _tmpdir=None):
    x = np.asarray(x, dtype=np.float32)
    shared = make_host_inputs(
        np.asarray(w_dw), np.asarray(w_pw),
        np.asarray(g1), np.asarray(b1), np.asarray(m1), np.asarray(v1),
        np.asarray(g2), np.asarray(b2), np.asarray(m2), np.asarray(v2),
    )
    in_maps = []
    for i in range(N_CORES):
        m = {"x": np.ascontiguousarray(x[i * BS : (i + 1) * BS])}
        m.update(shared)
        in_maps.append(m)

    nc = _get_nc()
    res = run_bass_kernel_spmd(
        nc, in_maps, core_ids=list(range(N_CORES)), trace=_trace, tmpdir=_tmpdir
    )
    y = np.concatenate([res.results[i]["y"] for i in range(N_CORES)], axis=0)
    if _trace:
        return y, res
    return y
